# revision 1
# baseline (speedup 1.0000x reference)
"""CLUES loss (focal CE + supervised contrastive) on 8 Trainium2 NeuronCores.

Sharding: batch rows across cores; each core computes its [1024, 8192] slice
of the sim matrix and reduces it to two partial scalars; host sums 8 pairs.

Math notes (per core, rows R = its 1024-row slice):
  contrastive_i = (P_i*log(S_i + 1e-8) - masked_i) / max(P_i, 1)
    S_i      = sum_j exp(sim_ij)            (the only thing needing the N^2 block)
    masked_i = (e_i . class_sum[t_i] - ||e_i||^2) / T   (tiny matmuls instead)
    P_i      = count[t_i] - 1
  The big block is bf16 matmuls into PSUM; ACT applies exp in place with
  accum_out producing row sums for free.  Normalized bf16 embeddings are
  transposed via a DRAM round trip through the hardware DMA transpose
  (startup chunks transpose on the then-idle PE instead).
"""

import sys

if '/opt/trn_rl_repo' not in sys.path:
    sys.path.insert(0, '/opt/trn_rl_repo')

import numpy as np

import concourse.bass as bass
import concourse.mybir as mybir
import concourse.tile as tile
from concourse.vector_clock import ScopedClock
from concourse.bass_utils import run_bass_kernel_spmd

F32 = mybir.dt.float32
BF16 = mybir.dt.bfloat16
I32 = mybir.dt.int32
AF = mybir.ActivationFunctionType
ALU = mybir.AluOpType
AX = mybir.AxisListType

N, C, D = 8192, 8, 256
NCORES = 8
ROWS = N // NCORES            # 1024 rows per core
NT = N // 128                 # 64 row tiles of the full batch
MT = ROWS // 128              # 8 row tiles per core
GAMMA = 2.0
LS = 0.1
ALPHA = 0.3
TEMP = 0.07
INV_T = 1.0 / TEMP

# column chunking of the 8192-wide sim block (psum big tiles are [128, 1536]);
# small leading chunks shorten the startup ramp of the chunk pipeline
CHUNKS = [512, 512, 1024, 1536, 1536, 1536, 1536]
assert sum(CHUNKS) == N
NCH = len(CHUNKS)


# ---------------------------------------------------------------------------
# walrus in this container only accepts ONE semaphore wait per instruction,
# while Tile freely attaches several.  Patch 1 fixes the final drain; patch 2
# is a post-pass hoisting extra waits onto same-engine NoOp carriers.
# ---------------------------------------------------------------------------

def _patched_drain_and_barrier(self, tick_clock, wait_clock):
    nc = self.nc
    carrier = nc.sync.nop(nofuse=True, hint="drain_wait_carrier")
    wait_clock.add_sem_waits(carrier.ins, ScopedClock({None: tick_clock.global_clock}))
    si = carrier.ins.sync_info
    waits = list(si.on_wait or []) if si is not None else []
    if len(waits) > 1:
        carrier.ins.sync_info = mybir.SyncInfo(
            on_wait=waits[:1], on_update=list(si.on_update or []))
        for w in waits[1:]:
            n2 = nc.sync.nop(nofuse=True, hint="drain_wait_carrier")
            n2.ins.sync_info = mybir.SyncInfo(on_wait=[w], on_update=[])
    nc.sync.drain()
    nc.all_engine_barrier()
    popped = nc._tile_sem_poison_stack.pop()
    assert popped is self._sem_poison
    nc.clear_and_free_semaphores(list(self.sems.allocated().values()))
    nc.all_engine_barrier()


tile.TileContext._drain_and_barrier = _patched_drain_and_barrier


def _split_multi_waits(nc):
    """One sem wait per instruction: move extras to NoOp carriers just before."""
    n_split = 0
    for f in nc.m.functions:
        for bb in f.blocks:
            new = []
            for inst in bb.instructions:
                si = inst.sync_info
                waits = list(si.on_wait or []) if si is not None else []
                if len(waits) > 1:
                    for w in waits[:-1]:
                        nop = mybir.InstNoOp(
                            name=f"{inst.name}-wsplit{n_split}",
                            engine=inst.engine,
                            bass_nofuse=True,
                            sync_info=mybir.SyncInfo(on_wait=[w], on_update=[]),
                        )
                        n_split += 1
                        new.append(nop)
                    inst.sync_info = mybir.SyncInfo(
                        on_wait=[waits[-1]], on_update=list(si.on_update or []))
                new.append(inst)
            bb.instructions[:] = new


# ---------------------------------------------------------------------------
# kernel build
# ---------------------------------------------------------------------------

def _build():
    nc = bass.Bass()

    xf_d = nc.dram_tensor("xf", [N, D], F32, kind="ExternalInput")
    xo_d = nc.dram_tensor("xo", [ROWS, D], F32, kind="ExternalInput")
    lg_d = nc.dram_tensor("lg", [ROWS, C], F32, kind="ExternalInput")
    tf_d = nc.dram_tensor("tf", [N], F32, kind="ExternalInput")
    to_d = nc.dram_tensor("to", [ROWS], F32, kind="ExternalInput")
    out_d = nc.dram_tensor("out", [1, 2], F32, kind="ExternalOutput")

    xn_scr = nc.dram_tensor("xn_scr", [N, D], BF16, kind="Internal")
    xo_scr = nc.dram_tensor("xo_scr", [ROWS, D], BF16, kind="Internal")

    ch_t0 = np.cumsum([0] + CHUNKS) // 128  # chunk boundaries in 128-row tiles

    with tile.TileContext(nc) as tc:
        with (
            tc.tile_pool(name="big", bufs=1) as big,
            tc.tile_pool(name="work", bufs=3) as work,
            tc.tile_pool(name="pss", bufs=1, space="PSUM") as pss,
            tc.tile_pool(name="psc", bufs=1, space="PSUM") as psc,
            tc.tile_pool(name="psb", bufs=2, space="PSUM") as psb,
        ):
            # ============ phase 0: loads ====================================
            # prime the ACT exp/ln table load at t=0 (no data dependency) so
            # the first real activation doesn't pay the ~1.3us table DMA
            warm = big.tile([128, 1], F32, tag="warm")
            nc.vector.memset(warm, 0.0)
            nc.scalar.activation(out=warm, in_=warm, func=AF.Exp)
            # heavy, critical-path loads first on the HWDGE (sync) queue
            xo = big.tile([128, MT, D], F32, tag="xo")
            xo_ap = xo_d.ap().rearrange("(t p) d -> p t d", p=128)
            nc.sync.dma_start(out=xo[:, 0:2, :], in_=xo_ap[:, 0:2, :])
            nc.sync.dma_start(out=xo[:, 2:MT, :], in_=xo_ap[:, 2:MT, :])
            xf = big.tile([128, NT, D], F32, tag="xf")
            xf_ap = xf_d.ap().rearrange("(t p) d -> p t d", p=128)
            for _ci in (0, 1):
                _t0, _t1 = int(ch_t0[_ci]), int(ch_t0[_ci + 1])
                nc.sync.dma_start(out=xf[:, _t0:_t1, :], in_=xf_ap[:, _t0:_t1, :])
            # small inputs on the second HWDGE ring (scalar), all with
            # contiguous per-partition access patterns.
            # CE-side tensors use their own row mapping (row = p*MT + m);
            # the focal sum is row-order invariant.
            lg = big.tile([128, MT, C], F32, tag="lg")
            nc.scalar.dma_start(out=lg, in_=lg_d.ap().rearrange("(p t) c -> p t c", p=128))
            to_rm = big.tile([128, MT], F32, tag="to_rm")
            nc.scalar.dma_start(out=to_rm, in_=to_d.ap().rearrange("(p t) -> p t", p=128))
            t_b8 = big.tile([8, ROWS], F32, tag="t_b8")
            nc.scalar.dma_start(out=t_b8, in_=bass.AP(tensor=to_d.ap().tensor, offset=0,
                                                      ap=[[0, 8], [1, ROWS]]))
            # targets in the sim-row mapping (row = t*128 + p): contiguous
            # load of the transposed view + a PE transpose
            t_flat = big.tile([64, 128], F32, tag="t_flat")
            nc.scalar.dma_start(out=t_flat, in_=tf_d.ap().rearrange("(t p) -> t p", p=128))
            ident = big.tile([64, 64], F32, tag="ident")
            from concourse.masks import make_identity
            make_identity(nc, ident)
            ident16 = big.tile([128, 128], BF16, tag="ident16")
            make_identity(nc, ident16)
            t_rm_ps = pss.tile([128, 64], F32, tag="small")
            nc.tensor.transpose(out=t_rm_ps, in_=t_flat, identity=ident)
            t_rm = big.tile([128, NT], F32, tag="t_rm")
            nc.vector.tensor_copy(out=t_rm, in_=t_rm_ps)

            # ============ phase 1: own rows -> xoT (lhsT for everything) =====
            s2o = big.tile([128, MT], F32, tag="s2o")
            r_own = big.tile([128, MT], F32, tag="r_own")
            for t in range(MT):
                sq = work.tile([128, D], F32, tag="sqscr")
                nc.scalar.activation(out=sq, in_=xo[:, t, :], func=AF.Square,
                                     accum_out=s2o[:, t:t + 1])
            lnso = work.tile([128, MT], F32, tag="m2o")
            nc.scalar.activation(out=lnso, in_=s2o, func=AF.Ln)
            nc.scalar.activation(out=r_own, in_=lnso, func=AF.Exp, scale=-0.5)

            xo16 = big.tile([128, MT, D], BF16, tag="xo16")
            for t in range(MT):
                nc.vector.tensor_scalar(out=xo16[:, t, :], in0=xo[:, t, :],
                                        scalar1=r_own[:, t:t + 1], scalar2=None,
                                        op0=ALU.mult, op1=ALU.bypass)
            nc.sync.dma_start(out=xo_scr.ap().rearrange("(t p) d -> p t d", p=128),
                              in_=xo16)
            xoT_a = big.tile([128, ROWS], BF16, tag="xoT_a")
            xoT_b = big.tile([128, ROWS], BF16, tag="xoT_b")
            nc.sync.dma_start_transpose(out=xoT_a, in_=xo_scr.ap()[:, 0:128])
            nc.sync.dma_start_transpose(out=xoT_b, in_=xo_scr.ap()[:, 128:256])

            # ============ phase 2: focal CE (emitted after first chunk) ======
            focal = big.tile([128, MT], F32, tag="focal")

            def emit_ce():
              mx = big.tile([128, MT], F32, tag="mx")
              nc.vector.reduce_max(out=mx, in_=lg, axis=AX.X)
              nmx = big.tile([128, MT], F32, tag="nmx")
              nc.vector.tensor_scalar(out=nmx, in0=mx, scalar1=-1.0, scalar2=None,
                                      op0=ALU.mult, op1=ALU.bypass)
              sumexp = big.tile([128, MT], F32, tag="sumexp")
              for m in range(MT):
                  esc = work.tile([128, C], F32, tag="esc")
                  nc.scalar.activation(out=esc, in_=lg[:, m, :], func=AF.Exp,
                                       bias=nmx[:, m:m + 1], scale=1.0,
                                       accum_out=sumexp[:, m:m + 1])
              logZ = big.tile([128, MT], F32, tag="logZ")
              nc.scalar.activation(out=logZ, in_=sumexp, func=AF.Ln)
              nc.vector.tensor_tensor(out=logZ, in0=logZ, in1=mx, op=ALU.add)
              xt = big.tile([128, MT], F32, tag="xt")
              ohl = work.tile([128, MT, C], F32, tag="ohl")
              nc.vector.tensor_tensor(out=ohl, in0=lg, in1=oh_own, op=ALU.mult)
              nc.vector.reduce_sum(out=xt, in_=ohl, axis=AX.X)
              sx = big.tile([128, MT], F32, tag="sx")
              nc.vector.reduce_sum(out=sx, in_=lg, axis=AX.X)
              ce = big.tile([128, MT], F32, tag="ce")
              u1 = work.tile([128, MT], F32, tag="u1")
              nc.vector.tensor_scalar(out=u1, in0=xt, scalar1=1.0 - LS, scalar2=None,
                                      op0=ALU.mult, op1=ALU.bypass)
              u2 = work.tile([128, MT], F32, tag="u2")
              nc.vector.tensor_scalar(out=u2, in0=sx, scalar1=LS / C, scalar2=None,
                                      op0=ALU.mult, op1=ALU.bypass)
              nc.vector.tensor_tensor(out=u1, in0=u1, in1=u2, op=ALU.add)
              nc.vector.tensor_tensor(out=ce, in0=logZ, in1=u1, op=ALU.subtract)
              pt_t = work.tile([128, MT], F32, tag="pt")
              nc.scalar.activation(out=pt_t, in_=ce, func=AF.Exp, scale=-1.0)
              nc.vector.tensor_scalar(out=pt_t, in0=pt_t, scalar1=-1.0, scalar2=1.0,
                                      op0=ALU.mult, op1=ALU.add)
              nc.vector.tensor_tensor(out=focal, in0=pt_t, in1=pt_t, op=ALU.mult)
              nc.vector.tensor_tensor(out=focal, in0=focal, in1=ce, op=ALU.mult)

            # ============ phase 3: per-chunk pipeline ========================
            mv = big.tile([128, NT, 2], F32, tag="mv")
            s2 = big.tile([128, NT], F32, tag="s2")
            r_all = big.tile([128, NT], F32, tag="r_all")
            xn16 = big.tile([128, NT, D + 1], BF16, tag="xn16")
            nc.vector.memset(xn16[:, :, D:D + 1], 1.0)
            xT_a = big.tile([128, N], BF16, tag="xT_a")
            xT_b = big.tile([128, N], BF16, tag="xT_b")
            Sacc = big.tile([128, MT, NCH], F32, tag="Sacc")
            cs_ps = psc.tile([8, D + 1], F32, tag="cs")

            xn_ap = xn_scr.ap().rearrange("(t p) d -> p t d", p=128)

            def load_chunk(ci):
                t0, t1 = int(ch_t0[ci]), int(ch_t0[ci + 1])
                nc.sync.dma_start(out=xf[:, t0:t1, :], in_=xf_ap[:, t0:t1, :])

            def norm_chunk(ci):
                """stats -> r -> bf16 scale -> DRAM round trip -> xT."""
                t0, t1 = int(ch_t0[ci]), int(ch_t0[ci + 1])
                nt_c = t1 - t0
                csz = CHUNKS[ci]
                rows0 = t0 * 128
                for t in range(t0, t1):
                    st = work.tile([128, 6], F32, tag="bnst")
                    nc.vector.bn_stats(out=st, in_=xf[:, t, :])
                    nc.vector.bn_aggr(out=mv[:, t, :], in_=st)
                # r = 1/sqrt((var + mean^2) * D)
                m2 = work.tile([128, nt_c], F32, tag="m2")
                nc.vector.tensor_tensor(out=m2, in0=mv[:, t0:t1, 0],
                                        in1=mv[:, t0:t1, 0], op=ALU.mult)
                nc.vector.tensor_tensor(out=s2[:, t0:t1], in0=m2,
                                        in1=mv[:, t0:t1, 1], op=ALU.add)
                nc.vector.tensor_scalar(out=s2[:, t0:t1], in0=s2[:, t0:t1],
                                        scalar1=float(D), scalar2=None,
                                        op0=ALU.mult, op1=ALU.bypass)
                lns = work.tile([128, nt_c], F32, tag="lns")
                nc.scalar.activation(out=lns, in_=s2[:, t0:t1], func=AF.Ln)
                nc.scalar.activation(out=r_all[:, t0:t1], in_=lns,
                                     func=AF.Exp, scale=-0.5)
                for t in range(t0, t1):
                    nc.vector.tensor_scalar(out=xn16[:, t, :D], in0=xf[:, t, :],
                                            scalar1=r_all[:, t:t + 1], scalar2=None,
                                            op0=ALU.mult, op1=ALU.bypass)
                if ci <= 2:
                    # startup chunks: PE/ACT are idle here — transpose on the
                    # PE and skip the DRAM round trip + its DMA queue slots
                    for h, xT_h in ((0, xT_a), (1, xT_b)):
                        tp_ps = psb.tile([128, csz], BF16, tag="bigps")
                        for t in range(t0, t1):
                            nc.tensor.transpose(
                                out=tp_ps[:, (t - t0) * 128:(t - t0 + 1) * 128],
                                in_=xn16[:, t, h * 128:(h + 1) * 128],
                                identity=ident16)
                        nc.vector.tensor_copy(out=xT_h[:, rows0:rows0 + csz], in_=tp_ps)
                else:
                    nc.sync.dma_start(out=xn_ap[:, t0:t1, :], in_=xn16[:, t0:t1, :D])
                    nc.sync.dma_start_transpose(
                        out=xT_a[:, rows0:rows0 + csz],
                        in_=xn_scr.ap()[rows0:rows0 + csz, 0:128])
                    nc.sync.dma_start_transpose(
                        out=xT_b[:, rows0:rows0 + csz],
                        in_=xn_scr.ap()[rows0:rows0 + csz, 128:256])

            def main_chunk(ci):
                """class-sum partials + sim-block matmuls + in-place exp."""
                t0, t1 = int(ch_t0[ci]), int(ch_t0[ci + 1])
                csz = CHUNKS[ci]
                rows0 = t0 * 128
                for m in range(MT):
                    pt = psb.tile([128, 1536], F32, tag="bigps")
                    la = xoT_a[:, m * 128:(m + 1) * 128]
                    lb = xoT_b[:, m * 128:(m + 1) * 128]
                    for n0 in range(0, csz, 512):
                        nc.tensor.matmul(pt[:, n0:n0 + 512], la,
                                         xT_a[:, rows0 + n0:rows0 + n0 + 512],
                                         start=True, stop=False)
                        nc.tensor.matmul(pt[:, n0:n0 + 512], lb,
                                         xT_b[:, rows0 + n0:rows0 + n0 + 512],
                                         start=False, stop=True)
                    nc.scalar.activation(out=pt[:, :csz], in_=pt[:, :csz],
                                         func=AF.Exp, scale=INV_T,
                                         accum_out=Sacc[:, m, ci:ci + 1])
                for t in range(t0, t1):
                    nc.tensor.matmul(cs_ps, oh_rm[:, t, :], xn16[:, t, :],
                                     start=(t == 0), stop=(t == NT - 1))

            # software-pipelined: loads run 2 chunks ahead and normalize runs
            # 1 chunk ahead of the main compute, so the DMA FIFO never queues
            # a chunk's load behind the previous chunk's (compute-gated)
            # write-back, and the tiny ACT ops of the next chunk's normalize
            # clear the ACT queue before the bulk exps.
            norm_chunk(0)  # chunks 0 and 1 were loaded back in phase 0
            norm_chunk(1)

            def emit_onehots():
                iota8_i = big.tile([128, C], I32, tag="iota8i")
                nc.gpsimd.iota(iota8_i, pattern=[[1, C]], base=0, channel_multiplier=0)
                iota8 = big.tile([128, C], F32, tag="iota8")
                nc.vector.tensor_copy(out=iota8, in_=iota8_i)
                iotac_i = big.tile([8, 1], I32, tag="iotaci")
                nc.gpsimd.iota(iotac_i, pattern=[[0, 1]], base=0, channel_multiplier=1)
                iotac = big.tile([8, 1], F32, tag="iotac")
                nc.vector.tensor_copy(out=iotac, in_=iotac_i)

                oh_rm = big.tile([128, NT, C], BF16, tag="oh_rm")
                nc.vector.tensor_tensor(
                    out=oh_rm,
                    in0=t_rm.to_broadcast([128, NT, C]),
                    in1=bass.AP(tensor=iota8.tensor, offset=iota8.offset,
                                ap=[iota8.ap[0], [0, NT], iota8.ap[1]]),
                    op=ALU.is_equal)
                oh_own = big.tile([128, MT, C], F32, tag="oh_own")
                nc.vector.tensor_tensor(
                    out=oh_own,
                    in0=to_rm.to_broadcast([128, MT, C]),
                    in1=bass.AP(tensor=iota8.tensor, offset=iota8.offset,
                                ap=[iota8.ap[0], [0, MT], iota8.ap[1]]),
                    op=ALU.is_equal)
                ohT_own = big.tile([8, ROWS], BF16, tag="ohT_own")
                nc.vector.tensor_scalar(out=ohT_own, in0=t_b8, scalar1=iotac,
                                        scalar2=None, op0=ALU.is_equal, op1=ALU.bypass)
                return oh_rm, oh_own, ohT_own


            for ci in range(NCH):
                if ci + 2 < NCH:
                    load_chunk(ci + 2)
                if ci + 2 < NCH:
                    norm_chunk(ci + 2)
                if ci == 0:
                    # one-hot writers must precede main_chunk(0)'s class-sum
                    # reads in emission order, but sit late in the DVE FIFO so
                    # the (slow, strided) target loads can't block normalize
                    oh_rm, oh_own, ohT_own = emit_onehots()
                main_chunk(ci)
                if ci == 0:
                    emit_ce()

            # ============ phase 4: gathers + contrastive assembly ============
            cs16 = big.tile([8, D + 1], BF16, tag="cs16")
            nc.vector.tensor_copy(out=cs16, in_=cs_ps)
            Mp = big.tile([128, MT], F32, tag="Mp")
            cnt = big.tile([128, MT], F32, tag="cnt")
            for m in range(MT):
                sel_ps = pss.tile([128, D + 1], F32, tag="small")
                nc.tensor.matmul(sel_ps, ohT_own[:, m * 128:(m + 1) * 128], cs16,
                                 start=True, stop=True)
                scr = work.tile([128, D], F32, tag="selscr")
                nc.vector.tensor_tensor(out=scr, in0=xo[:, m, :],
                                        in1=sel_ps[:, :D], op=ALU.mult)
                nc.vector.reduce_sum(out=Mp[:, m:m + 1], in_=scr, axis=AX.X)
                nc.vector.tensor_copy(out=cnt[:, m:m + 1], in_=sel_ps[:, D:D + 1])

            S = big.tile([128, MT], F32, tag="S")
            nc.vector.reduce_sum(out=S, in_=Sacc, axis=AX.X)
            logS = big.tile([128, MT], F32, tag="logS")
            eps_t = big.tile([128, 1], F32, tag="eps_t")
            nc.vector.memset(eps_t, 1e-8)
            nc.scalar.activation(out=logS, in_=S, func=AF.Ln, bias=eps_t)

            # con = (P*logS - (Mp*r - s2o*r^2)*invT) / max(P,1)
            P = big.tile([128, MT], F32, tag="P")
            nc.vector.tensor_scalar(out=P, in0=cnt, scalar1=-1.0, scalar2=None,
                                    op0=ALU.add, op1=ALU.bypass)
            npos = work.tile([128, MT], F32, tag="npos")
            nc.vector.tensor_scalar(out=npos, in0=P, scalar1=1.0, scalar2=None,
                                    op0=ALU.max, op1=ALU.bypass)
            rinv = work.tile([128, MT], F32, tag="rinv")
            nc.vector.reciprocal(out=rinv, in_=npos)
            t2 = work.tile([128, MT], F32, tag="t2")
            nc.vector.tensor_tensor(out=t2, in0=Mp, in1=r_own, op=ALU.mult)
            rr = work.tile([128, MT], F32, tag="rr")
            nc.vector.tensor_tensor(out=rr, in0=r_own, in1=r_own, op=ALU.mult)
            nc.vector.tensor_tensor(out=rr, in0=rr, in1=s2o, op=ALU.mult)
            nc.vector.tensor_tensor(out=t2, in0=t2, in1=rr, op=ALU.subtract)
            nc.vector.tensor_scalar(out=t2, in0=t2, scalar1=INV_T, scalar2=None,
                                    op0=ALU.mult, op1=ALU.bypass)
            t1_ = work.tile([128, MT], F32, tag="t1")
            nc.vector.tensor_tensor(out=t1_, in0=P, in1=logS, op=ALU.mult)
            con = big.tile([128, MT], F32, tag="con")
            nc.vector.tensor_tensor(out=con, in0=t1_, in1=t2, op=ALU.subtract)
            nc.vector.tensor_tensor(out=con, in0=con, in1=rinv, op=ALU.mult)

            # ============ phase 5: reduce to [1, 2] and store ================
            colbuf = big.tile([128, 2], F32, tag="colbuf")
            nc.vector.reduce_sum(out=colbuf[:, 0:1], in_=focal, axis=AX.X)
            nc.vector.reduce_sum(out=colbuf[:, 1:2], in_=con, axis=AX.X)
            ones = big.tile([128, 1], F32, tag="ones")
            nc.vector.memset(ones, 1.0)
            fin_ps = pss.tile([1, 2], F32, tag="small")
            nc.tensor.matmul(fin_ps, ones, colbuf, start=True, stop=True)
            out_sb = big.tile([1, 2], F32, tag="out_sb")
            nc.vector.tensor_copy(out=out_sb, in_=fin_ps)
            nc.sync.dma_start(out=out_d.ap(), in_=out_sb)

    _split_multi_waits(nc)
    return nc


_NC = None
LAST_RESULTS = None  # BassKernelResults of the most recent run (for profiling)
RUN_KWARGS = {}      # extra kwargs for run_bass_kernel_spmd (e.g. trace=True)


def _get_nc():
    global _NC
    if _NC is None:
        _NC = _build()
    return _NC


def kernel(logits, embeddings, targets):
    logits = np.ascontiguousarray(np.asarray(logits), dtype=np.float32)
    embeddings = np.ascontiguousarray(np.asarray(embeddings), dtype=np.float32)
    targets_np = np.asarray(targets)
    tf32 = np.ascontiguousarray(targets_np.astype(np.float32))

    nc = _get_nc()
    in_maps = []
    for k in range(NCORES):
        sl = slice(k * ROWS, (k + 1) * ROWS)
        in_maps.append({
            "xf": embeddings,
            "xo": np.ascontiguousarray(embeddings[sl]),
            "lg": np.ascontiguousarray(logits[sl]),
            "tf": tf32,
            "to": np.ascontiguousarray(tf32[sl]),
        })
    res = run_bass_kernel_spmd(nc, in_maps, core_ids=list(range(NCORES)), **RUN_KWARGS)
    global LAST_RESULTS
    LAST_RESULTS = res

    fsum = 0.0
    csum = 0.0
    for k in range(NCORES):
        o = res.results[k]["out"]
        fsum += float(o[0, 0])
        csum += float(o[0, 1])
    ce_loss = np.float32(fsum / N)
    con_loss = np.float32(csum / N)
    total = np.float32(ce_loss + np.float32(ALPHA) * con_loss)
    return (total, ce_loss, con_loss)

